# revision 20
# baseline (speedup 1.0000x reference)
"""Tensor-parallel GQA attention block (dense_transformer) on 8 TRN2 NeuronCores.

Sharding: tensor parallel across heads. Core c gets q-heads 4c..4c+3 and
kv-head c (GQA groups intact). Instead of a row-parallel wo + AllReduce,
each core AllGathers the (already softmax-normalized) per-head attention
outputs y and computes a 512-column slice of the output projection; the
host concatenates slices (cheaper: AG of 4MB/rank vs AR of 32MB/rank).

Device-side layouts (prepared on host):
  - everything matmul-facing is stored "contraction dim on partitions":
    xT [DIM, S], w*T [DIM, out], woT [DIM, 512-slice].
  - q/k head dims are permuted so RoPE pairs (2i, 2i+1) sit 16 partitions
    apart inside one 32-partition quadrant => the rotation's half-swap is a
    single DVE stream_shuffle. Scores are invariant to the (common) perm.
  - softmax scale 1/sqrt(hd) is folded into wq on the host.
  - causal mask is applied from 4 host-built additive [128, 512] tiles on
    the diagonal band only; strictly-above-diagonal tiles are skipped.

Numerics: matmuls run as float32r (TF32-like, ~2.5e-4 rel err measured);
softmax skips max-subtraction (scores are O(10), exp can't overflow f32).
"""

import ml_dtypes
import numpy as np

import concourse.bass as bass
import concourse.mybir as mybir
import concourse.tile as tile
from concourse import bacc
from concourse.bass_utils import run_bass_kernel_spmd

F32 = mybir.dt.float32
F32R = mybir.dt.float32r
BF16 = mybir.dt.bfloat16
AF = mybir.ActivationFunctionType

N_CORES = 8
DIM = 4096
S = 2048
HEAD_DIM = 128
N_HEADS = 32
N_KV = 8
HPC = N_HEADS // N_CORES        # q heads per core = 4
P = 128
SC = 512                        # seq chunk (free dim of most matmuls)
N_SCHUNK = S // SC              # 4
N_KTILE = DIM // P              # 32
N_STILE = S // P                # 16

SWAP16 = list(range(16, 32)) + list(range(16))   # per-quadrant 16-rotation


def build(debug_taps: bool = False):
    nc = bacc.Bacc(None, num_devices=N_CORES)

    xT = nc.declare_dram_parameter("xT", [DIM, S], F32R, isOutput=False)
    # fused qkv weights: [:, 0:512] q heads, [:, 512:640] k, [:, 640:768] v
    wqkvT = nc.declare_dram_parameter("wqkvT", [DIM, 768], F32R, isOutput=False)
    woT = nc.declare_dram_parameter("woT", [DIM, SC], BF16, isOutput=False)
    cosd = nc.declare_dram_parameter("cosd", [P, S], F32, isOutput=False)
    sins = nc.declare_dram_parameter("sins", [P, S], F32, isOutput=False)
    cmask = nc.declare_dram_parameter("cmask", [P, 4 * SC], F32, isOutput=False)
    out = nc.dram_tensor("out", [S, SC], F32, kind="ExternalOutput")

    taps = {}
    if debug_taps:
        taps["qt"] = nc.dram_tensor("qt", [P, HPC, S], F32, kind="ExternalOutput")
        taps["kt"] = nc.dram_tensor("kt", [P, S], F32, kind="ExternalOutput")
        taps["vv"] = nc.dram_tensor("vv", [P, N_STILE, HEAD_DIM], F32, kind="ExternalOutput")
        taps["dd"] = nc.dram_tensor("dd", [HPC, S], F32, kind="ExternalOutput")
        taps["yl"] = nc.dram_tensor("yl", [P, HPC, S], BF16, kind="ExternalOutput")

    with tile.TileContext(nc) as tc:
        # PSUM bank plan ([128, 512] f32 banks):
        #   proj:    psq0-3 -> banks0-3, psk -> E, psv -> F, V-transp -> G/H
        #   attn:    pss -> banks0-3 (4-deep), psy -> E/F, psd -> G/H
        #   outproj: pso -> E/F
        ps = tc.alloc_tile_pool(name="ps", bufs=1, space="PSUM")
        const = tc.alloc_tile_pool(name="const", bufs=1)
        pw = tc.alloc_tile_pool(name="pw", bufs=1, side="right")
        main = tc.alloc_tile_pool(name="main", bufs=1)
        stream = tc.alloc_tile_pool(name="stream", bufs=3)
        tmp = tc.alloc_tile_pool(name="tmp", bufs=2)
        dram = tc.alloc_tile_pool(name="dram", bufs=1, space="DRAM")

        # ---- constants ---------------------------------------------------
        ones_f = const.tile([P, P], F32)
        nc.vector.memset(ones_f[:], 1.0)
        ones = const.tile([P, P], BF16)
        nc.scalar.copy(ones[:], ones_f[:])
        ident = const.tile([P, P], F32)
        from concourse.masks import make_identity
        make_identity(nc, ident[:])
        mask_sb = const.tile([P, 4 * SC], F32)
        nc.sync.dma_start(mask_sb[:], cmask[:])
        cos_sb = pw.tile([P, S], F32)
        sin_sb = pw.tile([P, S], F32)
        nc.sync.dma_start(cos_sb[:], cosd[:])
        nc.sync.dma_start(sin_sb[:], sins[:])
        wo_sb = pw.tile([P, N_KTILE, SC], BF16)
        nc.scalar.dma_start(wo_sb[:], woT.rearrange("(t p) m -> p t m", p=P))

        kt_sb = main.tile([P, S], F32R)
        v_sb = main.tile([P, N_STILE, HEAD_DIM], BF16)

        ybounce = [
            dram.tile([HPC * P, SC], BF16, name=f"ybounce{ci}")
            for ci in range(N_SCHUNK)
        ]
        ygather = [
            dram.tile([N_CORES * HPC * P, SC], BF16, addr_space="Shared",
                      name=f"ygather{ci}")
            for ci in range(N_SCHUNK)
        ]

        # ---- per-chunk segment emitters ----------------------------------
        def proj(ci):
            s_lo = ci * SC
            qtc = tmp.tile([P, HPC, SC], F32R, tag="qtc", name=f"qtc{ci}")
            qt_tiles[ci] = qtc
            psq = [
                ps.tile([P, SC], F32, tag=f"bank{h}", name=f"psq{h}_{ci}")
                for h in range(HPC)
            ]
            psk = ps.tile([P, SC], F32, tag="bankE", name=f"psk{ci}")
            psv = ps.tile([P, SC], F32, tag="bankF", name=f"psv{ci}")
            for k in range(N_KTILE):
                xs = stream.tile([P, SC], F32R, tag="xs", bufs=5, name=f"xs{ci}_{k}")
                nc.sync.dma_start(xs[:], xT[k * P:(k + 1) * P, s_lo:s_lo + SC])
                wt = stream.tile([P, 768], F32R, tag="wt", bufs=4, name=f"wt{ci}_{k}")
                nc.scalar.dma_start(wt[:], wqkvT[k * P:(k + 1) * P, :])
                st = dict(start=(k == 0), stop=(k == N_KTILE - 1))
                for h in range(HPC):
                    nc.tensor.matmul(
                        psq[h][:], wt[:, h * P:(h + 1) * P], xs[:], **st
                    )
                nc.tensor.matmul(psk[:], wt[:, 512:640], xs[:], **st)
                nc.tensor.matmul(psv[:], wt[:, 640:768], xs[:], **st)

            # rope: out = psum*cos + shuffle16(psum)*sins
            for h in range(HPC + 1):
                src = psk if h == HPC else psq[h]
                dst = kt_sb[:, s_lo:s_lo + SC] if h == HPC \
                    else qtc[:, h, :]
                raw = tmp.tile([P, SC], F32, tag="rraw", name=f"rr{ci}_{h}")
                nc.scalar.copy(raw[:], src[:])
                qc = tmp.tile([P, SC], F32, tag="rqc", name=f"rq{ci}_{h}")
                nc.vector.tensor_mul(qc[:], src[:], cos_sb[:, s_lo:s_lo + SC])
                qsw = tmp.tile([P, SC], F32, tag="rqs", name=f"rs{ci}_{h}")
                nc.vector.stream_shuffle(qsw[:], raw[:], SWAP16)
                nc.vector.tensor_mul(qsw[:], qsw[:], sin_sb[:, s_lo:s_lo + SC])
                nc.vector.tensor_add(dst, qc[:], qsw[:])

            # V: evict then transpose to [s, hd] (bf16)
            vt = tmp.tile([P, SC], F32, tag="vt", name=f"vt{ci}")
            nc.scalar.copy(vt[:], psv[:])
            for q in range(SC // P):
                pst = ps.tile(
                    [P, P], F32, tag=("bankG" if q % 2 == 0 else "bankH"),
                    name=f"pst{ci}_{q}",
                )
                nc.tensor.transpose(pst[:], vt[:, q * P:(q + 1) * P], ident[:])
                nc.vector.tensor_copy(v_sb[:, ci * 4 + q, :], pst[:])

        def attn(ci):
            s_lo = ci * SC
            n_j = 4 * ci + 4
            for h in range(HPC):
                psy = ps.tile(
                    [P, SC], F32, tag=("bankE" if h % 2 == 0 else "bankF"),
                    name=f"psy{h}_{ci}",
                )
                psd = ps.tile(
                    [1, SC], F32, tag=("bankG" if h % 2 == 0 else "bankH"),
                    name=f"psd{h}_{ci}",
                )
                for t in range(n_j):
                    pss = ps.tile(
                        [P, SC], F32, tag=f"bank{t % 4}", name=f"pss{h}_{ci}_{t}"
                    )
                    nc.tensor.matmul(
                        pss[:],
                        kt_sb[:, t * P:(t + 1) * P],
                        qt_tiles[ci][:, h, :],
                        start=True, stop=True,
                    )
                    pt = tmp.tile([P, SC], BF16, tag="pt", bufs=4,
                                  name=f"pt{h}_{ci}_{t}")
                    d = t - 4 * ci
                    if d >= 0:
                        ms = tmp.tile([P, SC], F32, tag="ms", name=f"ms{h}_{ci}_{t}")
                        nc.vector.tensor_add(
                            ms[:], pss[:], mask_sb[:, d * SC:(d + 1) * SC]
                        )
                        nc.scalar.activation(pt[:], ms[:], AF.Exp)
                    else:
                        nc.scalar.activation(pt[:], pss[:], AF.Exp)
                    st = dict(start=(t == 0), stop=(t == n_j - 1))
                    nc.tensor.matmul(psy[:], v_sb[:, t, :], pt[:], **st)
                    nc.tensor.matmul(psd[:], ones[:, 0:1], pt[:], **st)

                # normalize off the PE
                dsb = tmp.tile([1, SC], F32, tag="dsb", name=f"dsb{h}_{ci}")
                nc.scalar.copy(dsb[:], psd[:])
                rc1 = tmp.tile([1, SC], F32, tag="rc1", name=f"rc1{h}_{ci}")
                nc.vector.reciprocal_approx_fast(rc1[:], dsb[:])
                rbb = tmp.tile([P, SC], F32, tag="rbb", name=f"rbb{h}_{ci}")
                nc.gpsimd.partition_broadcast(rbb[:], rc1[:])
                yp = tmp.tile([P, SC], BF16, tag="yp", name=f"yp{h}_{ci}")
                nc.vector.tensor_mul(yp[:], psy[:], rbb[:])
                nc.gpsimd.dma_start(ybounce[ci][h * P:(h + 1) * P, :], yp[:])
                if debug_taps:
                    nc.sync.dma_start(taps["yl"][:, h, s_lo:s_lo + SC], yp[:])
                    nc.sync.dma_start(
                        taps["dd"][h:h + 1, s_lo:s_lo + SC], dsb[:]
                    )

            nc.gpsimd.collective_compute(
                "AllGather",
                mybir.AluOpType.bypass,
                replica_groups=[list(range(N_CORES))],
                ins=[ybounce[ci][:]],
                outs=[ygather[ci][:]],
            )

        def outproj(ci):
            g_lo = ci * SC
            for st_i in range(4):
                pso = ps.tile(
                    [P, SC], F32, tag=("bankE" if st_i % 2 == 0 else "bankF"),
                    name=f"pso{ci}_{st_i}",
                )
                if st_i == 0:
                    ygc = stream.tile([P, N_KTILE, SC], BF16, tag="yg", bufs=1,
                                      name=f"yg{ci}")
                    nc.sync.dma_start(
                        ygc[:], ygather[ci].rearrange("(t p) m -> p t m", p=P)
                    )
                    yg_tiles[ci] = ygc
                for kt in range(N_KTILE):
                    nc.tensor.matmul(
                        pso[:], yg_tiles[ci][:, kt, st_i * P:(st_i + 1) * P],
                        wo_sb[:, kt, :],
                        start=(kt == 0), stop=(kt == N_KTILE - 1),
                    )
                ob = tmp.tile([P, SC], F32, tag="ob", name=f"ob{ci}_{st_i}")
                nc.scalar.copy(ob[:], pso[:])
                nc.sync.dma_start(
                    out[g_lo + st_i * P:g_lo + (st_i + 1) * P, :], ob[:]
                )

        # ---- software-pipelined emission ---------------------------------
        yg_tiles = {}
        qt_tiles = {}
        proj(0)
        attn(0)
        proj(1)
        outproj(0)
        attn(1)
        proj(2)
        outproj(1)
        attn(2)
        proj(3)
        outproj(2)
        attn(3)
        outproj(3)

        if debug_taps:
            for ci in range(N_SCHUNK):
                nc.sync.dma_start(
                    taps["qt"][:, :, ci * SC:(ci + 1) * SC],
                    qt_tiles[ci][:].bitcast(F32),
                )
            nc.sync.dma_start(taps["kt"][:], kt_sb[:].bitcast(F32))
            nc.sync.dma_start(taps["vv"][:], v_sb[:])

        pw.release()
        for pool in (dram, tmp, stream, main, const, ps):
            pool.release()

    nc.compile()
    return nc


# ---------------------------------------------------------------------------
# host-side prep / unshard
# ---------------------------------------------------------------------------

def _perm128():
    """head-dim permutation: pair i=(16q+j) -> even at 32q+j, odd at 32q+16+j."""
    order = np.empty(128, dtype=np.int64)
    for i in range(64):
        q, j = i // 16, i % 16
        order[32 * q + j] = 2 * i
        order[32 * q + 16 + j] = 2 * i + 1
    return order


def _host_prep(x, freqs_cis, wq, wk, wv, wo):
    order = _perm128()
    xT = np.ascontiguousarray(x[0].T)                       # [DIM, S]
    scale = np.float32(1.0 / np.sqrt(HEAD_DIM))

    cosT = np.ascontiguousarray(freqs_cis[:, :, 0].T)       # [64, S]
    sinT = np.ascontiguousarray(freqs_cis[:, :, 1].T)
    cosd = np.empty((P, S), dtype=np.float32)
    sins = np.empty((P, S), dtype=np.float32)
    for q in range(4):
        cosd[32 * q:32 * q + 16] = cosT[16 * q:16 * q + 16]
        cosd[32 * q + 16:32 * q + 32] = cosT[16 * q:16 * q + 16]
        sins[32 * q:32 * q + 16] = -sinT[16 * q:16 * q + 16]
        sins[32 * q + 16:32 * q + 32] = sinT[16 * q:16 * q + 16]

    ii = np.arange(SC)[None, :]
    jj = np.arange(P)[:, None]
    cmask = np.empty((P, 4 * SC), dtype=np.float32)
    for d in range(4):
        cmask[:, d * SC:(d + 1) * SC] = np.where(
            ii >= jj + P * d, np.float32(0.0), np.float32(-1e9)
        )

    in_maps = []
    for c in range(N_CORES):
        wq_c = wq[c * 512:(c + 1) * 512].reshape(HPC, 128, DIM)[:, order, :]
        wq_c = (wq_c.reshape(512, DIM) * scale).astype(np.float32)
        wk_c = wk[c * 128:(c + 1) * 128][order]
        wv_c = wv[c * 128:(c + 1) * 128]
        wqkv_c = np.concatenate([wq_c, wk_c, wv_c], axis=0)
        wo_c = wo[c * 512:(c + 1) * 512]
        in_maps.append({
            "xT": xT,
            "wqkvT": np.ascontiguousarray(wqkv_c.T),
            "woT": np.ascontiguousarray(wo_c.T).astype(ml_dtypes.bfloat16),
            "cosd": cosd,
            "sins": sins,
            "cmask": cmask,
        })
    return in_maps


_NC_CACHE = {}


def get_nc(debug_taps=False):
    key = bool(debug_taps)
    if key not in _NC_CACHE:
        _NC_CACHE[key] = build(debug_taps=key)
    return _NC_CACHE[key]


def kernel(x, freqs_cis, mask, wq, wk, wv, wo, _trace=False, _debug_taps=False):
    in_maps = _host_prep(x, freqs_cis, wq, wk, wv, wo)
    nc = get_nc(_debug_taps)
    res = run_bass_kernel_spmd(
        nc, in_maps, core_ids=list(range(N_CORES)), trace=_trace
    )
    full = np.concatenate([res.results[c]["out"] for c in range(N_CORES)], axis=1)
    out = full.reshape(1, S, DIM).astype(np.float32)
    if _trace or _debug_taps:
        kernel.last_results = res
    return out
